# revision 2
# baseline (speedup 1.0000x reference)
"""DeformConv1d Trainium2 Bass kernel.

Problem: x[4,512,4096] f32, offsets[4,1,4090,7] f32, weight[512,512,7], bias[512]
  T[b,o,k]   = clamp(o + k + offsets[b,0,o,k], o, o+6)
  samp[b,c,o,k] = linear-interp of x[b,c,:] at T
  out[b,d,o] = sum_{c,k} samp[b,c,o,k] * weight[d,c,k] + bias[d]

Key identity: the clamp keeps every sample inside [o, o+6], so output o only
touches taps p in [o, o+7], and the interpolation weight of tap p is exactly
relu(1 - |p - T|).  With an o-tile of 121 the tap band is exactly 128 wide:

  out[o,d] = sum_{k, dp<128} S_k[dp, o] * Y[o0+dp, k, d]
    S_k[dp,o] = relu(1 - |(dp - (o-o0)) - c_k[o]|),  c_k[o] = clamp(k + off, 0, 6)
    Y[p,k,d]  = sum_c x[c, p] * weight[d, c, k]

Both stages are dense bf16 matmuls on the PE array (f32 PSUM accumulate).
Because each S_k column sums to exactly 1 (the two interp taps straddle T),
the bias is folded into Y during PSUM eviction: Y'[p,k,d] = Y + bias[d]/5 for
k<5, which adds exactly bias[d] to every output -- no bias matmul on the PE.
The per-tile c rows are pre-tiled on the host into [TILES, 7*OT] so a single
DRAM staging pass + one partition-broadcast DMA per tile produces S's operand
(no per-tile DRAM->DRAM relayouts).  Weights are DMA'd in 28 (k,ci) chunks in
exact PE consumption order so the first matmul fires as soon as 128KB lands.
Sharding: 8 cores = 4 batches x 2 halves of out_len (data parallel, no comm).
"""

import os
import sys

import ml_dtypes
import numpy as np

for _p in ("/opt/trn_rl_repo", os.path.expanduser("~/.axon_site/_ro/trn_rl_repo")):
    if os.path.isdir(_p) and _p not in sys.path:
        sys.path.insert(0, _p)

import concourse.mybir as mybir
import concourse.tile as tile
from concourse import bacc
from concourse.bass_utils import run_bass_kernel_spmd

B, CIN, COUT, L, K = 4, 512, 512, 4096, 7
OUT_LEN = 4090
HALF = 2045          # out positions per core (2 halves per batch)
OT = 121             # o-tile size -> tap band = OT + 7 = 128
TILES = 17           # 17 * 121 = 2057 >= 2045
OPAD = TILES * OT    # 2057 padded out positions per core
XW = (TILES - 1) * OT + 128  # 2064: rightmost x column any tile reads
P = 128
NCK = CIN // P       # 4 c-chunks
SW = K * OT          # 847: flat width of one tile's c/S rows (k-major blocks)
F32 = mybir.dt.float32
BF16 = mybir.dt.bfloat16

_prog_cache = {}


def _build_program():
    nc = bacc.Bacc("TRN2", target_bir_lowering=False, debug=False)

    xs_d = nc.dram_tensor("xs", [CIN, XW], BF16, kind="ExternalInput")
    wt_d = nc.dram_tensor("wt", [CIN, K, COUT], BF16, kind="ExternalInput")
    offs3_d = nc.dram_tensor("offs3", [TILES, SW], F32, kind="ExternalInput")
    bias5_d = nc.dram_tensor("bias5", [1, COUT], F32, kind="ExternalInput")
    diag_d = nc.dram_tensor("diag7", [P, SW], F32, kind="ExternalInput")
    kpat_d = nc.dram_tensor("kpat", [1, SW], F32, kind="ExternalInput")
    out_d = nc.dram_tensor("out", [OPAD, COUT], F32, kind="ExternalOutput")

    with tile.TileContext(nc) as tc:
        with (
            tc.tile_pool(name="const", bufs=1) as cpool,
            tc.tile_pool(name="cdram", bufs=1, space="DRAM") as dpool,
            tc.tile_pool(name="cbt", bufs=4) as cbpool,
            tc.tile_pool(name="stiles", bufs=3) as stpool,
            tc.tile_pool(name="ytiles", bufs=3) as ypool,
            tc.tile_pool(name="otiles", bufs=3) as opool,
            tc.tile_pool(name="psy", bufs=5, space="PSUM") as psy,
            tc.tile_pool(name="pso", bufs=3, space="PSUM") as pso,
        ):
            # ---- small consts / offsets on the gpsimd queue, in the order
            # their consumers fire (bias -> first Y evict ~11us, then S chain)
            bias_sb = cpool.tile([P, COUT], F32)
            nc.gpsimd.dma_start(bias_sb[:], bias5_d[:].partition_broadcast(P))
            kpat = cpool.tile([TILES, SW], F32)
            nc.gpsimd.dma_start(kpat[:], kpat_d[:].partition_broadcast(TILES))
            c17 = cpool.tile([TILES, SW], F32)
            nc.gpsimd.dma_start(c17[:], offs3_d[:])
            diag7 = cpool.tile([P, SW], F32)
            nc.gpsimd.dma_start(diag7[:], diag_d[:])

            # c[t, k*OT+j] = clamp(k + off, 0, 6) for the whole core at once,
            # staged to DRAM so a 0-stride DMA can expand rows across
            # partitions per tile
            nc.vector.tensor_tensor(c17[:], c17[:], kpat[:], mybir.AluOpType.add)
            nc.vector.tensor_scalar(
                c17[:], c17[:], 0.0, 6.0, mybir.AluOpType.max, mybir.AluOpType.min,
            )
            c_dram = dpool.tile([TILES, SW], F32)
            nc.gpsimd.dma_start(c_dram[:], c17[:])

            # ---- bulk inputs on the sync queue, in PE consumption order:
            # x band for tile 0, then W in 28 (k,ci) chunks, then the rest of x
            xs = cpool.tile([P, NCK, XW], BF16)
            wt = cpool.tile([P, NCK, K, COUT], BF16)
            XCUTS = [0, 130, 258, 775, 1420, XW]
            xs_src = xs_d[:].rearrange("(ci p) t -> p ci t", p=P)
            wt_src = wt_d[:].rearrange("(ci p) k d -> p ci k d", p=P)
            nc.sync.dma_start(xs[:, :, 0:130], xs_src[:, :, 0:130])
            for k in range(K):
                for ci in range(NCK):
                    nc.sync.dma_start(wt[:, ci, k, :], wt_src[:, ci, k, :])
            for lo, hi in zip(XCUTS[1:], XCUTS[2:]):
                nc.sync.dma_start(xs[:, :, lo:hi], xs_src[:, :, lo:hi])

            for t in range(TILES):
                o0 = t * OT
                last = t == TILES - 1

                # ---- Y[dp, k, d] for band p in [o0, o0+128) ----
                y_sb = ypool.tile([P, K, COUT], BF16, tag="y_sb")
                for k in range(K):
                    yp = psy.tile([P, COUT], F32, tag="yp")
                    for ci in range(NCK):
                        nc.tensor.matmul(
                            yp[:],
                            xs[:, ci, o0 : o0 + P],
                            wt[:, ci, k, :],
                            start=(ci == 0), stop=(ci == NCK - 1),
                        )
                    # bias/5 folded into taps k<5 (S columns sum to 1)
                    if k < 5:
                        nc.vector.tensor_tensor(
                            y_sb[:, k, :], yp[:], bias_sb[:], mybir.AluOpType.add,
                        )
                    else:
                        nc.scalar.copy(y_sb[:, k, :], yp[:])

                # ---- S_k[dp, o] = relu(1 - |(dp - j) - c_k|) ----
                # expand this tile's c row across all 128 partitions with a
                # 0-stride DMA (3.4KB contiguous per partition)
                cb = cbpool.tile([P, SW], F32, tag="cb")
                s_sb = stpool.tile([P, SW], BF16, tag="s_sb")
                nc.gpsimd.dma_start(
                    cb[:], c_dram[t : t + 1, :].partition_broadcast(P)
                )
                nc.vector.tensor_tensor(
                    cb[:], cb[:], diag7[:], mybir.AluOpType.subtract,
                )
                nc.scalar.activation(
                    cb[:], cb[:], mybir.ActivationFunctionType.Abs,
                )
                nc.scalar.activation(
                    s_sb[:], cb[:],
                    mybir.ActivationFunctionType.Relu,
                    bias=1.0, scale=-1.0,
                )

                # ---- out[o, d] = sum_k S_k^T Y_k (bias already in Y) ----
                op = pso.tile([P, COUT], F32, tag="op")
                if not last:
                    for k in range(K):
                        nc.tensor.matmul(
                            op[:OT],
                            s_sb[:, k * OT : (k + 1) * OT],
                            y_sb[:, k, :],
                            start=(k == 0), stop=(k == K - 1),
                        )
                    o_sb = opool.tile([P, COUT], F32, tag="o_sb")
                    if t % 2 == 0:
                        nc.scalar.copy(o_sb[:OT], op[:OT])
                    else:
                        nc.vector.tensor_copy(o_sb[:OT], op[:OT])
                    nc.sync.dma_start(out_d[o0 : o0 + OT, :], o_sb[:OT])
                else:
                    # last tile: split d in halves so the final evict+DMA
                    # overlaps the second half's matmuls (shorter tail)
                    HC = COUT // 2
                    o_sb = opool.tile([P, COUT], F32, tag="o_sb")
                    for h, (eng, lo) in enumerate(
                        ((nc.vector.tensor_copy, 0), (nc.scalar.copy, HC))
                    ):
                        for k in range(K):
                            nc.tensor.matmul(
                                op[:OT, lo : lo + HC],
                                s_sb[:, k * OT : (k + 1) * OT],
                                y_sb[:, k, lo : lo + HC],
                                start=(k == 0), stop=(k == K - 1),
                            )
                        eng(o_sb[:OT, lo : lo + HC], op[:OT, lo : lo + HC])
                        nc.sync.dma_start(
                            out_d[o0 : o0 + OT, lo : lo + HC],
                            o_sb[:OT, lo : lo + HC],
                        )

    nc.compile()
    return nc


def _install_axon_ntff_hook():
    """Provide antenv.axon_hooks (absent on this image) so that
    run_bass_kernel_spmd(trace=True) can capture NTFF profiles via the
    axon .so's C ABI.  Mirrors trn_agent_boot.trn_boot."""
    import contextlib
    import ctypes
    import types

    try:
        from antenv.axon_hooks import set_axon_ntff_profile_hook  # noqa: F401
        return
    except ImportError:
        pass

    so_path = "/opt/axon/libaxon_pjrt.so"
    if not os.path.exists(so_path):
        return
    lib = ctypes.CDLL(so_path)
    if not hasattr(lib, "axon_start_nrt_profile"):
        return
    lib.axon_start_nrt_profile.argtypes = [
        ctypes.POINTER(ctypes.c_int64), ctypes.c_size_t,
    ]
    lib.axon_start_nrt_profile.restype = ctypes.c_int64
    lib.axon_stop_nrt_profile.argtypes = [ctypes.c_char_p]
    lib.axon_stop_nrt_profile.restype = ctypes.c_int64

    @contextlib.contextmanager
    def _hook(output_dir, device_ids):
        import jax

        jax.devices()
        if device_ids:
            ids = (ctypes.c_int64 * len(device_ids))(*device_ids)
            rc = lib.axon_start_nrt_profile(ids, len(device_ids))
        else:
            rc = lib.axon_start_nrt_profile(None, 0)
        if rc != 0:
            raise RuntimeError(f"axon_start_nrt_profile rc={rc}")
        try:
            yield
        finally:
            n = lib.axon_stop_nrt_profile(str(output_dir).encode())
            print(f"ntff profile: {n} file(s) written to {output_dir}")

    box = {"h": _hook}
    mod = types.ModuleType("antenv.axon_hooks")
    mod.get_axon_ntff_profile_hook = lambda: box["h"]
    mod.set_axon_ntff_profile_hook = lambda h: box.__setitem__("h", h)
    import antenv

    sys.modules["antenv.axon_hooks"] = mod
    antenv.axon_hooks = mod

    # zero-egress env: skip the artifact upload in the trace path
    from concourse import bass_utils as _bu

    _bu.upload_artifacts = lambda d: f"local:{d}"


def _consts():
    # diag7[dp, k*OT+j] = dp - j   (j = o - o0), flat k-major layout
    dp = np.arange(P, dtype=np.float32).reshape(P, 1)
    j = np.arange(OT, dtype=np.float32).reshape(1, OT)
    blk = dp - j  # [P, OT]
    diag7 = np.zeros((P, SW), dtype=np.float32)
    kpat = np.zeros((1, SW), dtype=np.float32)
    for k in range(K):
        diag7[:, k * OT : (k + 1) * OT] = blk
        kpat[0, k * OT : (k + 1) * OT] = k
    return diag7, kpat


def kernel(x, offsets, weight, bias, _trace=False, _trace_kwargs=None):
    x = np.asarray(x, dtype=np.float32)
    offsets = np.asarray(offsets, dtype=np.float32)
    weight = np.asarray(weight, dtype=np.float32)
    bias = np.asarray(bias, dtype=np.float32)

    if "nc" not in _prog_cache:
        _prog_cache["nc"] = _build_program()
    nc = _prog_cache["nc"]

    w_t = np.ascontiguousarray(
        np.transpose(weight, (1, 2, 0)).astype(ml_dtypes.bfloat16)
    )  # [c, k, d]
    bias5 = np.ascontiguousarray((bias / 5.0).reshape(1, COUT).astype(np.float32))
    diag7, kpat = _consts()

    in_maps = []
    for core in range(8):
        b, half = core // 2, core % 2
        o_off = half * HALF
        xs = np.zeros((CIN, XW), dtype=ml_dtypes.bfloat16)
        xw = min(L - o_off, XW)
        xs[:, :xw] = x[b][:, o_off : o_off + xw].astype(ml_dtypes.bfloat16)
        # offsets pre-tiled: row t holds k-major blocks [off_k(o0+j)]_j
        offs_pad = np.zeros((OPAD, K), dtype=np.float32)
        ow = min(OUT_LEN - o_off, OPAD)
        offs_pad[:ow] = offsets[b, 0, o_off : o_off + ow, :]
        offs3 = np.ascontiguousarray(
            offs_pad.reshape(TILES, OT, K).transpose(0, 2, 1).reshape(TILES, SW)
        )
        in_maps.append(
            {
                "xs": xs, "wt": w_t, "offs3": offs3, "bias5": bias5,
                "diag7": diag7, "kpat": kpat,
            }
        )

    if _trace:
        _install_axon_ntff_hook()
    try:
        res = run_bass_kernel_spmd(
            nc, in_maps, core_ids=list(range(8)),
            trace=_trace, **(_trace_kwargs or {}),
        )
    except Exception:
        # transient runtime faults have been observed; one retry
        res = run_bass_kernel_spmd(
            nc, in_maps, core_ids=list(range(8)),
            trace=_trace, **(_trace_kwargs or {}),
        )

    out = np.empty((B, COUT, OUT_LEN), dtype=np.float32)
    for core in range(8):
        b, half = core // 2, core % 2
        o_off = half * HALF
        out[b, :, o_off : o_off + HALF] = res.results[core]["out"][:HALF, :].T
    if _trace:
        _prog_cache["last_exec_time_ns"] = res.exec_time_ns
    return out


# revision 4
# speedup vs baseline: 1.0174x; 1.0174x over previous
"""DeformConv1d Trainium2 Bass kernel.

Problem: x[4,512,4096] f32, offsets[4,1,4090,7] f32, weight[512,512,7], bias[512]
  T[b,o,k]   = clamp(o + k + offsets[b,0,o,k], o, o+6)
  samp[b,c,o,k] = linear-interp of x[b,c,:] at T
  out[b,d,o] = sum_{c,k} samp[b,c,o,k] * weight[d,c,k] + bias[d]

Key identity: the clamp keeps every sample inside [o, o+6], so output o only
touches taps p in [o, o+7], and the interpolation weight of tap p is exactly
relu(1 - |p - T|).  With an o-tile of 121 the tap band is exactly 128 wide:

  out[o,d] = sum_{k, dp<128} S_k[dp, o] * Y[o0+dp, k, d]
    S_k[dp,o] = relu(1 - |(dp - (o-o0)) - c_k[o]|),  c_k[o] = clamp(k + off, 0, 6)
    Y[p,k,d]  = sum_c x[c, p] * weight[d, c, k]

Both stages are dense bf16 matmuls on the PE array (f32 PSUM accumulate).
Because each S_k column sums to exactly 1 (the two interp taps straddle T),
the bias is folded into Y during PSUM eviction: Y'[p,k,d] = Y + bias[d]/5 for
k<5, which adds exactly bias[d] to every output -- no bias matmul on the PE.
The per-tile c rows are pre-tiled on the host into [TILES, 7*OT] so a single
DRAM staging pass + one partition-broadcast DMA per tile produces S's operand
(no per-tile DRAM->DRAM relayouts).  Weights are DMA'd in 28 (k,ci) chunks in
exact PE consumption order so the first matmul fires as soon as 128KB lands.
Sharding: 8 cores = 4 batches x 2 halves of out_len (data parallel, no comm).
"""

import os
import sys

import ml_dtypes
import numpy as np

for _p in ("/opt/trn_rl_repo", os.path.expanduser("~/.axon_site/_ro/trn_rl_repo")):
    if os.path.isdir(_p) and _p not in sys.path:
        sys.path.insert(0, _p)

import concourse.mybir as mybir
import concourse.tile as tile
from concourse import bacc
from concourse.bass_utils import run_bass_kernel_spmd

B, CIN, COUT, L, K = 4, 512, 512, 4096, 7
OUT_LEN = 4090
HALF = 2045          # out positions per core (2 halves per batch)
OT = 121             # o-tile size -> tap band = OT + 7 = 128
TILES = 17           # 17 * 121 = 2057 >= 2045
OPAD = TILES * OT    # 2057 padded out positions per core
XW = (TILES - 1) * OT + 128  # 2064: rightmost x column any tile reads
P = 128
NCK = CIN // P       # 4 c-chunks
SW = K * OT          # 847: flat width of one tile's c/S rows (k-major blocks)
F32 = mybir.dt.float32
BF16 = mybir.dt.bfloat16

_prog_cache = {}


def _build_program():
    nc = bacc.Bacc("TRN2", target_bir_lowering=False, debug=False)

    xs_d = nc.dram_tensor("xs", [CIN, XW], BF16, kind="ExternalInput")
    wt_d = nc.dram_tensor("wt", [CIN, K, COUT], BF16, kind="ExternalInput")
    offs3_d = nc.dram_tensor("offs3", [TILES, SW], F32, kind="ExternalInput")
    bias5_d = nc.dram_tensor("bias5", [1, COUT], F32, kind="ExternalInput")
    diag_d = nc.dram_tensor("diag7", [P, SW], F32, kind="ExternalInput")
    kpat_d = nc.dram_tensor("kpat", [1, SW], F32, kind="ExternalInput")
    out_d = nc.dram_tensor("out", [OPAD, COUT], F32, kind="ExternalOutput")

    with tile.TileContext(nc) as tc:
        with (
            tc.tile_pool(name="const", bufs=1) as cpool,
            tc.tile_pool(name="cdram", bufs=1, space="DRAM") as dpool,
            tc.tile_pool(name="cbt", bufs=4) as cbpool,
            tc.tile_pool(name="stiles", bufs=3) as stpool,
            tc.tile_pool(name="ytiles", bufs=3) as ypool,
            tc.tile_pool(name="otiles", bufs=3) as opool,
            tc.tile_pool(name="psy", bufs=5, space="PSUM") as psy,
            tc.tile_pool(name="pso", bufs=3, space="PSUM") as pso,
        ):
            # ---- small consts / offsets on the gpsimd queue, in the order
            # their consumers fire (bias -> first Y evict ~11us, then S chain)
            bias_sb = cpool.tile([P, COUT], F32)
            nc.gpsimd.dma_start(bias_sb[:], bias5_d[:].partition_broadcast(P))
            kpat = cpool.tile([TILES, SW], F32)
            nc.gpsimd.dma_start(kpat[:], kpat_d[:].partition_broadcast(TILES))
            c17 = cpool.tile([TILES, SW], F32)
            nc.gpsimd.dma_start(c17[:], offs3_d[:])
            diag7 = cpool.tile([P, SW], F32)
            nc.gpsimd.dma_start(diag7[:], diag_d[:])

            # c[t, k*OT+j] = clamp(k + off, 0, 6) for the whole core at once,
            # staged to DRAM so a 0-stride DMA can expand rows across
            # partitions per tile
            nc.vector.tensor_tensor(c17[:], c17[:], kpat[:], mybir.AluOpType.add)
            nc.vector.tensor_scalar(
                c17[:], c17[:], 0.0, 6.0, mybir.AluOpType.max, mybir.AluOpType.min,
            )
            c_dram = dpool.tile([TILES, SW], F32)
            nc.gpsimd.dma_start(c_dram[:], c17[:])

            # ---- bulk inputs on the sync queue, in PE consumption order:
            # x band for tile 0, then W in 28 (k,ci) chunks, then the rest of x
            xs = cpool.tile([P, NCK, XW], BF16)
            wt = cpool.tile([P, NCK, K, COUT], BF16)
            XCUTS = [0, 130, 258, 775, 1420, XW]
            xs_src = xs_d[:].rearrange("(ci p) t -> p ci t", p=P)
            wt_src = wt_d[:].rearrange("(ci p) k d -> p ci k d", p=P)
            nc.sync.dma_start(xs[:, :, 0:130], xs_src[:, :, 0:130])
            for k in range(K):
                for ci in range(NCK):
                    nc.sync.dma_start(wt[:, ci, k, :], wt_src[:, ci, k, :])
            for lo, hi in zip(XCUTS[1:], XCUTS[2:]):
                nc.sync.dma_start(xs[:, :, lo:hi], xs_src[:, :, lo:hi])

            def stage1(t):
                # ---- Y[dp, k, d] for band p in [o0, o0+128) ----
                o0 = t * OT
                y_sb = ypool.tile([P, K, COUT], BF16, tag="y_sb")
                for k in range(K):
                    yp = psy.tile([P, COUT], F32, tag="yp")
                    for ci in range(NCK):
                        nc.tensor.matmul(
                            yp[:],
                            xs[:, ci, o0 : o0 + P],
                            wt[:, ci, k, :],
                            start=(ci == 0), stop=(ci == NCK - 1),
                        )
                    # bias/5 folded into taps k<5 (S columns sum to 1)
                    if k < 5:
                        nc.vector.tensor_tensor(
                            y_sb[:, k, :], yp[:], bias_sb[:], mybir.AluOpType.add,
                        )
                    else:
                        nc.scalar.copy(y_sb[:, k, :], yp[:])
                return y_sb

            def sbuild(t):
                # ---- S_k[dp, o] = relu(1 - |(dp - j) - c_k|) ----
                # expand this tile's c row across all 128 partitions with a
                # 0-stride DMA (3.4KB contiguous per partition)
                cb = cbpool.tile([P, SW], F32, tag="cb")
                s_sb = stpool.tile([P, SW], BF16, tag="s_sb")
                nc.gpsimd.dma_start(
                    cb[:], c_dram[t : t + 1, :].partition_broadcast(P)
                )
                nc.vector.tensor_tensor(
                    cb[:], cb[:], diag7[:], mybir.AluOpType.subtract,
                )
                nc.scalar.activation(
                    cb[:], cb[:], mybir.ActivationFunctionType.Abs,
                )
                nc.scalar.activation(
                    s_sb[:], cb[:],
                    mybir.ActivationFunctionType.Relu,
                    bias=1.0, scale=-1.0,
                )
                return s_sb

            def stage2(t, y_sb, s_sb):
                # ---- out[o, d] = sum_k S_k^T Y_k (bias already in Y) ----
                o0 = t * OT
                last = t == TILES - 1
                op = pso.tile([P, COUT], F32, tag="op")
                if not last:
                    for k in range(K):
                        nc.tensor.matmul(
                            op[:OT],
                            s_sb[:, k * OT : (k + 1) * OT],
                            y_sb[:, k, :],
                            start=(k == 0), stop=(k == K - 1),
                        )
                    o_sb = opool.tile([P, COUT], F32, tag="o_sb")
                    if t % 2 == 0:
                        nc.scalar.copy(o_sb[:OT], op[:OT])
                    else:
                        nc.vector.tensor_copy(o_sb[:OT], op[:OT])
                    nc.sync.dma_start(out_d[o0 : o0 + OT, :], o_sb[:OT])
                else:
                    # last tile: split d in halves so the final evict+DMA
                    # overlaps the second half's matmuls (shorter tail)
                    HC = COUT // 2
                    o_sb = opool.tile([P, COUT], F32, tag="o_sb")
                    for h, (eng, lo) in enumerate(
                        ((nc.vector.tensor_copy, 0), (nc.scalar.copy, HC))
                    ):
                        for k in range(K):
                            nc.tensor.matmul(
                                op[:OT, lo : lo + HC],
                                s_sb[:, k * OT : (k + 1) * OT],
                                y_sb[:, k, lo : lo + HC],
                                start=(k == 0), stop=(k == K - 1),
                            )
                        eng(o_sb[:OT, lo : lo + HC], op[:OT, lo : lo + HC])
                        nc.sync.dma_start(
                            out_d[o0 : o0 + OT, lo : lo + HC],
                            o_sb[:OT, lo : lo + HC],
                        )

            # software pipeline: build S(t) and run stage-1 of t+1 before
            # stage-2 of t, so the in-order PE/DVE/ACT streams always have a
            # full tile of slack for the S chain (bcast DMA -> sub -> abs ->
            # relu) and the PE never waits on it
            y_prev = stage1(0)
            s_prev = sbuild(0)
            for t in range(TILES):
                if t + 1 < TILES:
                    y_next = stage1(t + 1)
                    s_next = sbuild(t + 1)
                stage2(t, y_prev, s_prev)
                if t + 1 < TILES:
                    y_prev, s_prev = y_next, s_next

    nc.compile()
    return nc


def _install_axon_ntff_hook():
    """Provide antenv.axon_hooks (absent on this image) so that
    run_bass_kernel_spmd(trace=True) can capture NTFF profiles via the
    axon .so's C ABI.  Mirrors trn_agent_boot.trn_boot."""
    import contextlib
    import ctypes
    import types

    try:
        from antenv.axon_hooks import set_axon_ntff_profile_hook  # noqa: F401
        return
    except ImportError:
        pass

    so_path = "/opt/axon/libaxon_pjrt.so"
    if not os.path.exists(so_path):
        return
    lib = ctypes.CDLL(so_path)
    if not hasattr(lib, "axon_start_nrt_profile"):
        return
    lib.axon_start_nrt_profile.argtypes = [
        ctypes.POINTER(ctypes.c_int64), ctypes.c_size_t,
    ]
    lib.axon_start_nrt_profile.restype = ctypes.c_int64
    lib.axon_stop_nrt_profile.argtypes = [ctypes.c_char_p]
    lib.axon_stop_nrt_profile.restype = ctypes.c_int64

    @contextlib.contextmanager
    def _hook(output_dir, device_ids):
        import jax

        jax.devices()
        if device_ids:
            ids = (ctypes.c_int64 * len(device_ids))(*device_ids)
            rc = lib.axon_start_nrt_profile(ids, len(device_ids))
        else:
            rc = lib.axon_start_nrt_profile(None, 0)
        if rc != 0:
            raise RuntimeError(f"axon_start_nrt_profile rc={rc}")
        try:
            yield
        finally:
            n = lib.axon_stop_nrt_profile(str(output_dir).encode())
            print(f"ntff profile: {n} file(s) written to {output_dir}")

    box = {"h": _hook}
    mod = types.ModuleType("antenv.axon_hooks")
    mod.get_axon_ntff_profile_hook = lambda: box["h"]
    mod.set_axon_ntff_profile_hook = lambda h: box.__setitem__("h", h)
    import antenv

    sys.modules["antenv.axon_hooks"] = mod
    antenv.axon_hooks = mod

    # zero-egress env: skip the artifact upload in the trace path
    from concourse import bass_utils as _bu

    _bu.upload_artifacts = lambda d: f"local:{d}"


def _consts():
    # diag7[dp, k*OT+j] = dp - j   (j = o - o0), flat k-major layout
    dp = np.arange(P, dtype=np.float32).reshape(P, 1)
    j = np.arange(OT, dtype=np.float32).reshape(1, OT)
    blk = dp - j  # [P, OT]
    diag7 = np.zeros((P, SW), dtype=np.float32)
    kpat = np.zeros((1, SW), dtype=np.float32)
    for k in range(K):
        diag7[:, k * OT : (k + 1) * OT] = blk
        kpat[0, k * OT : (k + 1) * OT] = k
    return diag7, kpat


def kernel(x, offsets, weight, bias, _trace=False, _trace_kwargs=None):
    x = np.asarray(x, dtype=np.float32)
    offsets = np.asarray(offsets, dtype=np.float32)
    weight = np.asarray(weight, dtype=np.float32)
    bias = np.asarray(bias, dtype=np.float32)

    if "nc" not in _prog_cache:
        _prog_cache["nc"] = _build_program()
    nc = _prog_cache["nc"]

    w_t = np.ascontiguousarray(
        np.transpose(weight, (1, 2, 0)).astype(ml_dtypes.bfloat16)
    )  # [c, k, d]
    bias5 = np.ascontiguousarray((bias / 5.0).reshape(1, COUT).astype(np.float32))
    diag7, kpat = _consts()

    in_maps = []
    for core in range(8):
        b, half = core // 2, core % 2
        o_off = half * HALF
        xs = np.zeros((CIN, XW), dtype=ml_dtypes.bfloat16)
        xw = min(L - o_off, XW)
        xs[:, :xw] = x[b][:, o_off : o_off + xw].astype(ml_dtypes.bfloat16)
        # offsets pre-tiled: row t holds k-major blocks [off_k(o0+j)]_j
        offs_pad = np.zeros((OPAD, K), dtype=np.float32)
        ow = min(OUT_LEN - o_off, OPAD)
        offs_pad[:ow] = offsets[b, 0, o_off : o_off + ow, :]
        offs3 = np.ascontiguousarray(
            offs_pad.reshape(TILES, OT, K).transpose(0, 2, 1).reshape(TILES, SW)
        )
        in_maps.append(
            {
                "xs": xs, "wt": w_t, "offs3": offs3, "bias5": bias5,
                "diag7": diag7, "kpat": kpat,
            }
        )

    if _trace:
        _install_axon_ntff_hook()
    try:
        res = run_bass_kernel_spmd(
            nc, in_maps, core_ids=list(range(8)),
            trace=_trace, **(_trace_kwargs or {}),
        )
    except Exception:
        # transient runtime faults have been observed; one retry
        res = run_bass_kernel_spmd(
            nc, in_maps, core_ids=list(range(8)),
            trace=_trace, **(_trace_kwargs or {}),
        )

    out = np.empty((B, COUT, OUT_LEN), dtype=np.float32)
    for core in range(8):
        b, half = core // 2, core % 2
        o_off = half * HALF
        out[b, :, o_off : o_off + HALF] = res.results[core]["out"][:HALF, :].T
    if _trace:
        _prog_cache["last_exec_time_ns"] = res.exec_time_ns
    return out
